# revision 13
# baseline (speedup 1.0000x reference)
"""MultiHeadAttention forward on 8 Trainium2 NeuronCores.

Tensor-parallel over heads: each core owns 2 of 16 heads (d_loc=256 of the
2048 QKV output columns, and the matching 256 rows of Wo). Each core
computes a full-shape partial output; the host sums the 8 partials and
adds bo (+ bv @ Wo for the folded V bias).

Problem shape: x [2, 2048, 2048], 16 heads, d_k = 128; device math in
bf16 (tolerance 2e-2; bf16 lands ~5e-3) with fp32 PSUM accumulation.

PE cost model (measured): bf16 streams 0.5 cyc/row (2x the fp32r rate)
and every matmul pays a fixed ~120 ns serial LDWEIGHTS that neither
weight packing nor stationary reuse removes. So the kernel minimizes
MATMUL COUNT: attention matmuls write [128,1024] PSUM outputs (two
banks) so one score / one AV matmul covers a 1024-query chunk-pair per
k-tile, and V is projected transposed (512-wide streams like Q/K) then
DMA-XBAR-transposed into packed [128,128] natural tiles.

Softmax: scores transposed ST[tk, tq]; exp on ScalarE in [128,1024]
tiles; denominator = DVE bf16 accumulation of exp tiles + one
ones-matmul partition reduction per (head, chunk); reciprocal on DVE.

Emission order keeps the in-order PE fed while ScalarE paces attention:
proj(b0) | attn(b0)+proj(b1) fills | proj(b1) drain + outproj(b0) |
attn(b1)+outproj fills | tail.
"""

import functools
from collections import deque
from contextlib import ExitStack

import numpy as np

D_MODEL = 2048
NUM_HEADS = 16
DK = 128
B = 2
T = 2048
BT = B * T
N_CORES = 8
H_LOC = NUM_HEADS // N_CORES  # 2 heads per core
D_LOC = H_LOC * DK  # 256
C_TILES = D_MODEL // 128  # 16
TQ = 512  # tq chunk width (one PSUM bank in fp32)
NCH = T // TQ  # 4 chunks per batch
TK_TILES = T // 128  # 16
NDEST = 6  # q0 q1 k0 k1 v0 v1 projection destinations


def _body(ctx, tc, xT, wqkv, bqk, wo, y):
    import concourse.bass as bass  # noqa: F401
    from concourse import mybir

    nc = tc.nc
    f32 = mybir.dt.float32
    bf16 = mybir.dt.bfloat16
    Exp = mybir.ActivationFunctionType.Exp
    Add = mybir.AluOpType.add
    Bypass = mybir.AluOpType.bypass
    inv_sqrt_dk = 1.0 / float(np.sqrt(DK))

    # ---------------- resident tensors ----------------
    wpool = ctx.enter_context(tc.tile_pool(name="wpool", bufs=1))
    x_pool = ctx.enter_context(tc.tile_pool(name="x_pool", bufs=20))

    w_tiles = []
    xt_pre = []
    for i in range(C_TILES):
        xti = x_pool.tile([128, TQ], bf16, tag="xt", name=f"xtpre{i}")
        nc.sync.dma_start(out=xti, in_=xT[i * 128 : (i + 1) * 128, 0:TQ])
        xt_pre.append(xti)
        wt = wpool.tile([128, 3 * D_LOC], bf16, tag=f"w{i}", name=f"w{i}")
        nc.sync.dma_start(out=wt, in_=wqkv[i * 128 : (i + 1) * 128, :])
        w_tiles.append(wt)
    bqk_sb = wpool.tile([128, 4], f32, tag="bqk", name="bqk")
    nc.sync.dma_start(out=bqk_sb, in_=bqk[:, :])

    wo_tiles = []
    for d in range(2):
        wot = wpool.tile([128, D_MODEL], bf16, tag=f"wo{d}", name=f"wo{d}")
        nc.sync.dma_start(out=wot, in_=wo[d * 128 : (d + 1) * 128, :])
        wo_tiles.append(wot)

    ones = wpool.tile([128, 128], bf16, tag="ones", name="ones")
    nc.vector.memset(ones, 1.0)

    # ---------------- pools ----------------
    qkv_pool = ctx.enter_context(tc.tile_pool(name="qkv_pool", bufs=1))
    av_pool = ctx.enter_context(tc.tile_pool(name="av_pool", bufs=1))
    es_pool = ctx.enter_context(tc.tile_pool(name="es_pool", bufs=6))
    acc_pool = ctx.enter_context(tc.tile_pool(name="acc_pool", bufs=4))
    rc_pool = ctx.enter_context(tc.tile_pool(name="rc_pool", bufs=4))
    y_pool = ctx.enter_context(tc.tile_pool(name="y_pool", bufs=3))

    # PSUM (8 banks): ps_po 2x[128,512] + ps_s 2x[128,1024] = 4 + ps_av
    # 1x[128,1024] = 2.
    ps_po = ctx.enter_context(tc.tile_pool(name="ps_po", bufs=2, space="PSUM"))
    ps_s = ctx.enter_context(tc.tile_pool(name="ps_s", bufs=2, space="PSUM"))
    ps_av = ctx.enter_context(tc.tile_pool(name="ps_av", bufs=1, space="PSUM"))

    qT, kT, v_pk, avT = {}, {}, {}, {}

    def alloc_batch(b):
        qT[b] = [
            qkv_pool.tile([128, T], bf16, tag=f"qT{d}", name=f"qT{d}_{b}", bufs=2)
            for d in range(2)
        ]
        kT[b] = [
            qkv_pool.tile([128, T], bf16, tag=f"kT{d}", name=f"kT{d}_{b}", bufs=2)
            for d in range(2)
        ]
        # vT: V projected transposed like Q/K, then DMA-transposed into
        # packed natural tiles v_pk[t][h].
        vT = [
            qkv_pool.tile([128, T], bf16, tag=f"vT{d}", name=f"vT{d}_{b}", bufs=2)
            for d in range(2)
        ]
        v_pk[b] = [
            [
                qkv_pool.tile(
                    [128, 128], bf16, tag=f"v{t}_{h}", name=f"v{t}_{h}_{b}", bufs=2
                )
                for h in range(H_LOC)
            ]
            for t in range(TK_TILES)
        ]
        avT[b] = [
            av_pool.tile([128, T], bf16, tag=f"avT{d}", name=f"avT{d}_{b}", bufs=2)
            for d in range(2)
        ]
        return vT

    vT_b = {}
    xt_chunks = {}

    def emit_xt_dma(b, ch):
        t0 = b * T + ch * TQ
        xt = []
        for i in range(C_TILES):
            xti = x_pool.tile([128, TQ], bf16, tag="xt", name=f"xt{b}_{ch}_{i}")
            nc.sync.dma_start(
                out=xti, in_=xT[i * 128 : (i + 1) * 128, t0 : t0 + TQ]
            )
            xt.append(xti)
        return xt

    def finish_proj(b, ch, j, ps):
        # j -> (k0, k1, q0, q1, v0, v1); k first so attention deps clear
        # early. wqkv column order is q0 q1 k0 k1 v0 v1.
        dest = (kT[b][0], kT[b][1], qT[b][0], qT[b][1], vT_b[b][0], vT_b[b][1])[j]
        wcol = (2, 3, 0, 1, 4, 5)[j]
        sl = dest[:, ch * TQ : (ch + 1) * TQ]
        if j >= 4:
            nc.vector.tensor_copy(sl, ps)
            # transpose the finished 128-col blocks into packed natural tiles
            h = j - 4
            for ts in range(TQ // 128):
                t_idx = ch * (TQ // 128) + ts
                nc.sync.dma_start(
                    out=v_pk[b][t_idx][h],
                    in_=dest[:, t_idx * 128 : (t_idx + 1) * 128],
                    transpose=True,
                )
        else:
            # PSUM -> SBUF with per-partition bias add (q/k only)
            nc.vector.tensor_scalar_add(sl, ps, bqk_sb[:, wcol : wcol + 1])

    def proj_closures(b, ch, j):
        # one closure per accumulating matmul so attention can interleave
        st = {}
        wcol = (2, 3, 0, 1, 4, 5)[j]

        def mk(i):
            def go():
                if i == 0:
                    st["ps"] = ps_po.tile(
                        [128, TQ], f32, tag="po", name=f"psp{b}_{ch}_{j}"
                    )
                nc.tensor.matmul(
                    st["ps"],
                    w_tiles[i][:, wcol * 128 : (wcol + 1) * 128],
                    xt_chunks[(b, ch)][i],
                    start=(i == 0),
                    stop=(i == C_TILES - 1),
                )
                if i == C_TILES - 1:
                    finish_proj(b, ch, j, st["ps"])

            return go

        return [("pe", mk(i)) for i in range(C_TILES)]

    def o_t_closures(b, t):
        # output projection for one 128-row tile of y, as 4 closures of
        # ~2 matmuls each
        st = {}
        row0 = b * T + t * 128

        def mk(half, q):
            def go():
                if q == 0:
                    st[half] = y_pool.tile(
                        [128, D_MODEL // 2],
                        bf16,
                        tag="ystage",
                        name=f"ys{b}_{t}_{half}",
                    )
                nch_i = half * 2 + q
                ps = ps_po.tile(
                    [128, TQ], f32, tag="po", name=f"pso{b}_{t}_{nch_i}"
                )
                for d in range(2):
                    nc.tensor.matmul(
                        ps,
                        avT[b][d][:, t * 128 : (t + 1) * 128],
                        wo_tiles[d][:, nch_i * TQ : (nch_i + 1) * TQ],
                        start=(d == 0),
                        stop=(d == 1),
                    )
                nc.vector.tensor_copy(st[half][:, q * TQ : (q + 1) * TQ], ps)
                if q == 1:
                    nc.sync.dma_start(
                        out=y[
                            row0 : row0 + 128,
                            half * (D_MODEL // 2) : (half + 1) * (D_MODEL // 2),
                        ],
                        in_=st[half],
                    )

            return go

        return [("pe", mk(half, q)) for half in range(2) for q in range(2)]

    def emit_attn_unit(b, h, chp, fill_q, per_tk=2):
        # One unit covers the 1024-query chunk pair (2*chp, 2*chp+1).
        # fill_q is a deque of closures, each emitting ~one independent PE
        # matmul; a few are popped per k-tile so the in-order PE has work
        # while it waits on ScalarE's exp for the pav matmul.
        pav = ps_av.tile([128, 2 * TQ], f32, tag="av", name=f"pav{b}_{h}_{chp}")
        acc = [
            acc_pool.tile([128, TQ], bf16, tag="acc", name=f"acc{b}_{h}_{chp}_{c}")
            for c in range(2)
        ]
        q_sl = qT[b][h][:, chp * 2 * TQ : (chp + 1) * 2 * TQ]
        for tk in range(TK_TILES):
            pss = ps_s.tile(
                [128, 2 * TQ], f32, tag="s", name=f"pss{b}_{h}_{chp}_{tk}"
            )
            es = es_pool.tile(
                [128, 2 * TQ], bf16, tag="es", name=f"es{b}_{h}_{chp}_{tk}"
            )
            for c in range(2):
                nc.tensor.matmul(
                    pss[:, c * TQ : (c + 1) * TQ],
                    kT[b][h][:, tk * 128 : (tk + 1) * 128],
                    q_sl[:, c * TQ : (c + 1) * TQ],
                    start=True,
                    stop=True,
                )
            nc.scalar.activation(es, pss, Exp, scale=inv_sqrt_dk)
            # fills ride in the exp->pav latency window
            done = 0
            while done < per_tk and fill_q:
                kind, c = fill_q.popleft()
                c()
                if kind == "pe":
                    done += 1
            for c in range(2):
                nc.tensor.matmul(
                    pav[:, c * TQ : (c + 1) * TQ],
                    v_pk[b][tk][h],
                    es[:, c * TQ : (c + 1) * TQ],
                    start=(tk == 0),
                    stop=(tk == TK_TILES - 1),
                )
            with nc.allow_low_precision("softmax denominator partials, bf16"):
                if tk == 0:
                    nc.vector.tensor_copy(acc[0], es[:, :TQ])
                    nc.vector.tensor_copy(acc[1], es[:, TQ:])
                else:
                    nc.vector.tensor_add(acc[0], acc[0], es[:, :TQ])
                    nc.vector.tensor_add(acc[1], acc[1], es[:, TQ:])
        # a few extra fills cover the acc-chain drain before the denominators
        done = 0
        while done < 4 and fill_q:
            kind, c = fill_q.popleft()
            c()
            if kind == "pe":
                done += 1
        for c in range(2):
            ch = 2 * chp + c
            pdn = ps_po.tile([128, TQ], f32, tag="po", name=f"pdn{b}_{h}_{ch}")
            nc.tensor.matmul(pdn, ones[:, 0:128], acc[c], start=True, stop=True)
            rc = rc_pool.tile([128, TQ], f32, tag="rc", name=f"rc{b}_{h}_{ch}")
            nc.vector.reciprocal_approx_fast(out=rc, in_=pdn)
            nc.vector.tensor_mul(
                avT[b][h][:, ch * TQ : (ch + 1) * TQ],
                pav[:, c * TQ : (c + 1) * TQ],
                rc,
            )

    # ---------------- S1: projections for batch 0 ----------------
    vT_b[0] = alloc_batch(0)
    xt_chunks[(0, 0)] = xt_pre
    for ch in range(NCH):
        if ch + 1 < NCH:
            xt_chunks[(0, ch + 1)] = emit_xt_dma(0, ch + 1)  # prefetch
        for j in range(NDEST):
            for _, c in proj_closures(0, ch, j):
                c()

    # ---------------- S2: attn(b0) with proj(b1) fills ----------------
    vT_b[1] = alloc_batch(1)
    fq = deque()

    def dma_closure(b, ch):
        def go():
            xt_chunks[(b, ch)] = emit_xt_dma(b, ch)

        return ("free", go)

    fq.append(dma_closure(1, 0))
    fq.append(dma_closure(1, 1))  # one chunk of DMA lookahead
    for ch in range(NCH):
        if ch + 2 < NCH:
            fq.append(dma_closure(1, ch + 2))
        for j in range(NDEST):
            fq.extend(proj_closures(1, ch, j))

    for chp in range(NCH // 2):
        for h in range(H_LOC):
            emit_attn_unit(0, h, chp, fq, per_tk=1)

    # ---------------- S3: drain proj(b1), then outproj(b0) halves ------
    while fq:
        fq.popleft()[1]()
    for t in range(8):
        for _, c in o_t_closures(0, t):
            c()

    # ---------------- S4: attn(b1) with outproj fills ----------------
    oq = deque()
    for t in range(8, TK_TILES):
        oq.extend(o_t_closures(0, t))
    for chp in range(NCH // 2):
        for h in range(H_LOC):
            emit_attn_unit(1, h, chp, oq, per_tk=1)
            if chp == 0 and h == H_LOC - 1:
                for t in range(0, 8):
                    oq.extend(o_t_closures(1, t))
    for t in range(8, TK_TILES):
        oq.extend(o_t_closures(1, t))
    while oq:
        oq.popleft()[1]()


@functools.cache
def _build():
    from concourse import bacc
    import concourse.tile as tile
    from concourse import mybir

    nc = bacc.Bacc(
        "TRN2",
        target_bir_lowering=False,
        debug=False,
        enable_asserts=False,
        num_devices=N_CORES,
    )
    f32 = mybir.dt.float32
    bf16 = mybir.dt.bfloat16
    xT = nc.dram_tensor("xT", [D_MODEL, BT], bf16, kind="ExternalInput").ap()
    wqkv = nc.dram_tensor(
        "wqkv", [D_MODEL, 3 * D_LOC], bf16, kind="ExternalInput"
    ).ap()
    bqk = nc.dram_tensor("bqk", [128, 4], f32, kind="ExternalInput").ap()
    wo = nc.dram_tensor("wo", [D_LOC, D_MODEL], bf16, kind="ExternalInput").ap()
    y = nc.dram_tensor("y", [BT, D_MODEL], bf16, kind="ExternalOutput").ap()

    with tile.TileContext(nc) as tc:
        with ExitStack() as ctx:
            _body(ctx, tc, xT, wqkv, bqk, wo, y)
    nc.compile()
    return nc


def _shard_inputs(x, Wq, bq, Wk, bk, Wv, bv, Wo, bo):
    """Host-side sharding: returns per-core input maps."""
    import ml_dtypes

    bf = ml_dtypes.bfloat16
    f = np.float32
    xT = np.ascontiguousarray(
        np.asarray(x, f).reshape(BT, D_MODEL).T.astype(bf)
    )
    Wq, Wk, Wv, Wo = (np.asarray(a, f) for a in (Wq, Wk, Wv, Wo))
    bq, bk, bv = (np.asarray(a, f) for a in (bq, bk, bv))
    in_maps = []
    for c in range(N_CORES):
        sl = slice(c * D_LOC, (c + 1) * D_LOC)
        wqkv_pad = np.ascontiguousarray(
            np.concatenate([Wq[:, sl], Wk[:, sl], Wv[:, sl]], axis=1).astype(bf)
        )
        bqk_t = np.ascontiguousarray(
            np.stack(
                [
                    bq[sl][:128],
                    bq[sl][128:],
                    bk[sl][:128],
                    bk[sl][128:],
                ],
                axis=1,
            )
        )
        wo_loc = np.ascontiguousarray(Wo[sl, :].astype(bf))
        in_maps.append({"xT": xT, "wqkv": wqkv_pad, "bqk": bqk_t, "wo": wo_loc})
    return in_maps


def _run(in_maps, trace=False, **kwargs):
    from concourse.bass_utils import run_bass_kernel_spmd

    nc = _build()
    return run_bass_kernel_spmd(
        nc, in_maps, core_ids=list(range(N_CORES)), trace=trace, **kwargs
    )


def kernel(x, Wq, bq, Wk, bk, Wv, bv, Wo, bo):
    in_maps = _shard_inputs(x, Wq, bq, Wk, bk, Wv, bv, Wo, bo)
    res = _run(in_maps, trace=False)
    acc = np.zeros((BT, D_MODEL), np.float32)
    for rmap in res.results:
        acc += np.asarray(rmap["y"], dtype=np.float32)
    acc += np.asarray(bo, np.float32)[None, :]
    acc += (np.asarray(bv, np.float32) @ np.asarray(Wo, np.float32))[None, :]
    return acc.reshape(B, T, D_MODEL)
